# revision 1
# baseline (speedup 1.0000x reference)
"""Causal MHA (B=4, S=2048, D=1024, H=16, Dh=64) on 8 trn2 NeuronCores.

Sharding: core = (batch b = core//2) x (head-group g = core%2, 8 heads each).
No collectives: each core computes a partial output projection for its head
group; the host sums the two partials per batch.

On-chip layout is fully "transposed" (feature-major) so no on-chip transposes
are needed:
  - x^T [1024, 2048] is the input;  Q^T/K^T [512, 2048] come out of the
    projection with the moving operand = x^T.
  - RoPE pair-rotation is a fixed 128x128 matrix (folded per 2-head block)
    applied on the PE, plus two elementwise multiplies with cos/sin tables.
  - scores are computed directly as S^T [k, q] tiles (lhsT = K^T slice),
    softmax denominator comes for free from a ones-column appended to V.
  - attention output is O^T [d, q] (lhsT = V tile), which feeds the wo
    projection directly (lhsT = wo^T tiles).
Causality is exploited at tile granularity (only j*128 < qchunk_end k-tiles
are computed; the q-range of diagonal-band tiles is clipped; exact diagonal
128x128 blocks get a multiplicative 0/1 mask after exp).
"""
import os
from contextlib import ExitStack

import numpy as np
import ml_dtypes

import concourse.bass as bass
from concourse import bacc
import concourse.mybir as mybir
import concourse.tile as tile
from concourse.bass_utils import run_bass_kernel_spmd

BF16 = ml_dtypes.bfloat16
F32 = mybir.dt.float32
BF = mybir.dt.bfloat16

B, S, D, H, DH = 4, 2048, 1024, 16, 64
NG = 2               # head groups
HL = H // NG         # heads per core = 8
DG = HL * DH         # 512 local head dims
THETA = 10000.0
NDT = D // 128       # 8 d-tiles of x^T
NJT = DG // 128      # 4 tiles of Q^T/K^T/O^T rows
NST = S // 128       # 16 seq tiles
NSC = S // 512       # 4 seq chunks
EXPF = mybir.ActivationFunctionType.Exp


def _emit(tc, aps, reps=1):
    nc = tc.nc
    (xT, wqT, wkT, wvT, woT, ropeC, ropeS, rmat, cmask, out) = aps

    ctx = tc.ctx  # set by caller

    # ---------------- persistent SBUF residents ----------------
    singles = ctx.enter_context(tc.tile_pool(name="singles", bufs=1))
    wq_sb = singles.tile([128, NDT, DG], BF, tag="wq")
    wk_sb = singles.tile([128, NDT, DG], BF, tag="wk")
    wv_sb = singles.tile([128, NDT, DG], BF, tag="wv")
    wo_sb = singles.tile([128, NJT, D], BF, tag="wo")
    c_sb = singles.tile([128, S], F32, tag="ropec")
    s_sb = singles.tile([128, S], F32, tag="ropes")
    rm_sb = singles.tile([128, 128], BF, tag="rmat")
    msk_sb = singles.tile([128, 128], BF, tag="cmask")
    qt_sb = [singles.tile([128, S], BF, tag=f"qt{j}", name=f"qt{j}") for j in range(NJT)]
    kt_sb = [singles.tile([128, S], BF, tag=f"kt{j}", name=f"kt{j}") for j in range(NJT)]
    ot_sb = [singles.tile([128, S], BF, tag=f"ot{j}", name=f"ot{j}") for j in range(NJT)]
    v_sb = singles.tile([128, NST, 128 * HL], BF, tag="v")

    xpool = ctx.enter_context(tc.tile_pool(name="xstream", bufs=1))
    qpre_pool = ctx.enter_context(tc.tile_pool(name="qpre", bufs=4))
    tmp_pool = ctx.enter_context(tc.tile_pool(name="ropetmp", bufs=3))
    p_pool = ctx.enter_context(tc.tile_pool(name="ptiles", bufs=6))
    div_pool = ctx.enter_context(tc.tile_pool(name="div", bufs=2))
    out_pool = ctx.enter_context(tc.tile_pool(name="outc", bufs=3))

    # loads ordered so the first compute (V units, pair-0 proj) starts early
    def load_xt(sc):
        xt = xpool.tile([128, NDT, 512], BF, tag=f"xt{sc}", name=f"xt{sc}")
        nc.sync.dma_start(
            out=xt,
            in_=xT[:, sc * 512:(sc + 1) * 512].rearrange("(t p) w -> p t w", p=128),
        )
        return xt

    xt0 = xpool.tile([128, NDT, 512], BF, tag="xt0", name="xt0")
    for dt in range(NDT):
        nc.sync.dma_start(out=xt0[:, dt, :], in_=xT[dt * 128:(dt + 1) * 128, 0:512])
        nc.sync.dma_start(
            out=wv_sb[:, dt, :], in_=wvT[dt * 128:(dt + 1) * 128, :])
        nc.sync.dma_start(
            out=wq_sb[:, dt, :], in_=wqT[dt * 128:(dt + 1) * 128, :])
    xt_tiles = [xt0]
    nc.sync.dma_start(out=rm_sb, in_=rmat[:])
    nc.sync.dma_start(out=wk_sb, in_=wkT.rearrange("(t p) j -> p t j", p=128))
    nc.sync.dma_start(out=c_sb, in_=ropeC[:])
    nc.sync.dma_start(out=s_sb, in_=ropeS[:])
    for sc in range(1, NSC):
        xt_tiles.append(load_xt(sc))
    nc.sync.dma_start(out=msk_sb, in_=cmask[:])
    nc.sync.dma_start(out=wo_sb, in_=woT.rearrange("(t p) m -> p t m", p=128))
    # ones half-block per head: AV matmul then yields rowsum replicated
    # on out partitions 64..127 (no partition-broadcast needed for the div)
    nc.vector.memset(
        v_sb.rearrange("p s (h c) -> p s h c", h=HL)[:, :, :, 64:128], 1.0
    )

    for _rep in range(reps):
        _phases(nc, tc, ctx, locals())


def _phases(nc, tc, ctx, env):
    (xpool, qpre_pool, tmp_pool, p_pool, div_pool, out_pool) = (
        env["xpool"], env["qpre_pool"], env["tmp_pool"], env["p_pool"],
        env["div_pool"], env["out_pool"])
    (wq_sb, wk_sb, wv_sb, wo_sb, c_sb, s_sb, rm_sb, msk_sb) = (
        env["wq_sb"], env["wk_sb"], env["wv_sb"], env["wo_sb"], env["c_sb"],
        env["s_sb"], env["rm_sb"], env["msk_sb"])
    (qt_sb, kt_sb, ot_sb, v_sb, xT, out) = (
        env["qt_sb"], env["kt_sb"], env["ot_sb"], env["v_sb"], env["xT"],
        env["out"])

    with ExitStack() as ph:
        # 8 PSUM banks exactly: pp(2) + s0(1) + s1(1) + o0(2) + o1(2)
        psum_a = ph.enter_context(tc.tile_pool(name="psum_a", bufs=2, space="PSUM"))
        psum_s = ph.enter_context(tc.tile_pool(name="psum_s", bufs=2, space="PSUM"))
        psum_o = ph.enter_context(tc.tile_pool(name="psum_o", bufs=1, space="PSUM"))

        xt_tiles = env["xt_tiles"]

        # ---- projection unit builders (interleaved into attention) ----
        def proj_qk_mm(wsel, jt, sc):
            w_sb = wq_sb if wsel == 0 else wk_sb
            pp = psum_a.tile([128, 512], F32, tag="pp")
            for dt in range(NDT):
                nc.tensor.matmul(
                    pp, w_sb[:, dt, jt * 128:(jt + 1) * 128],
                    xt_tiles[sc][:, dt, :],
                    start=(dt == 0), stop=(dt == NDT - 1),
                )
            qpre = qpre_pool.tile([128, 512], BF, tag="qpre")
            nc.vector.tensor_copy(qpre, pp)
            return qpre

        def proj_qk_rope(qpre, wsel, pr, sc):
            dst = qt_sb if wsel == 0 else kt_sb
            rq = psum_a.tile([128, 512], F32, tag="pp")
            nc.tensor.matmul(rq, rm_sb, qpre, start=True, stop=True)
            t1 = tmp_pool.tile([128, 512], F32, tag="t1")
            t2 = tmp_pool.tile([128, 512], F32, tag="t2")
            cs = slice(sc * 512, (sc + 1) * 512)
            nc.vector.tensor_mul(t1, qpre, c_sb[:, cs])
            nc.vector.tensor_mul(t2, rq, s_sb[:, cs])
            nc.vector.tensor_add(dst[pr][:, cs], t1, t2)

        def make_pair_proj_closures(pr):
            clos = []
            for sc in range(NSC):
                for wsel in (0, 1):
                    def mk(wsel=wsel, sc=sc):
                        st = {}
                        def a():
                            st["qpre"] = proj_qk_mm(wsel, pr, sc)
                        def b():
                            proj_qk_rope(st["qpre"], wsel, pr, sc)
                        return a, b
                    a, b = mk()
                    clos.append(a)
                    clos.append(b)
            return clos

        def v_unit(sc, st4):
            st = sc * 4 + st4
            vp = psum_a.tile([128, 512], F32, tag="pp")
            for dt in range(NDT):
                nc.tensor.matmul(
                    vp, xt_tiles[sc][:, dt, st4 * 128:(st4 + 1) * 128],
                    wv_sb[:, dt, :],
                    start=(dt == 0), stop=(dt == NDT - 1),
                )
            nc.vector.tensor_copy(
                v_sb[:, st, :].rearrange("p (h c) -> p h c", h=HL)[:, :, 0:64],
                vp.rearrange("p (h c) -> p h c", h=HL),
            )

        def wo_unit(mt, sc):
            wp = psum_a.tile([128, 512], F32, tag="pp", name="wp")
            for jt in range(NJT):
                nc.tensor.matmul(
                    wp, wo_sb[:, jt, mt * 128:(mt + 1) * 128],
                    ot_sb[jt][:, sc * 512:(sc + 1) * 512],
                    start=(jt == 0), stop=(jt == NJT - 1),
                )
            ob = out_pool.tile([128, 512], F32, tag="ob", name="ob")
            nc.vector.tensor_copy(ob, wp)
            nc.sync.dma_start(
                out=out[mt * 128:(mt + 1) * 128, sc * 512:(sc + 1) * 512],
                in_=ob,
            )

        # ---- attention for one head pair, with proj closures woven in ----
        def attention(pr, weave):
            wi = 0          # closures popped
            ui = 0          # attention units emitted
            n_units = sum(2 * c + 4 for c in range(NSC))

            def pop_weave():
                nonlocal wi
                want = (ui * len(weave)) // max(1, n_units)
                while wi < min(want, len(weave)):
                    weave[wi]()
                    wi += 1

            h0, h1 = 2 * pr, 2 * pr + 1
            for c in range(NSC):
                jmax = 4 * c + 4
                o0 = psum_o.tile([128, 512], F32, tag="o0")
                o1 = psum_o.tile([128, 512], F32, tag="o1")
                # single-j units; 1-bank S tiles so o can double-buffer
                units = [(j,) for j in range(jmax)]

                def emit_s(u, hh, c=c, pr=pr):
                    tag = "s0" if hh == 0 else "s1"
                    rows = slice(0, 64) if hh == 0 else slice(64, 128)
                    sg = psum_s.tile([128, 1, 512], F32, tag=tag)
                    offs = []
                    for idx, j in enumerate(u):
                        off = max(0, j * 128 - c * 512)
                        w = 512 - off
                        qs = slice(c * 512 + off, (c + 1) * 512)
                        ks = slice(j * 128, (j + 1) * 128)
                        nc.tensor.matmul(
                            sg[:, idx, :w], kt_sb[pr][rows, ks],
                            qt_sb[pr][rows, qs], start=True, stop=True)
                        offs.append((off, w))
                    return sg, offs

                def consume(u, hh, sg, offs, o_ps, habs, c=c):
                    ptag = "p0" if hh == 0 else "p1"
                    pg = p_pool.tile([128, 1, 512], BF, tag=ptag)
                    if len(u) == 2:
                        nc.scalar.activation(pg, sg, EXPF, scale=0.125)
                    else:
                        off, w = offs[0]
                        nc.scalar.activation(
                            pg[:, 0, :w], sg[:, 0, :w], EXPF, scale=0.125)
                        if u[0] * 128 >= c * 512:
                            nc.vector.tensor_mul(
                                pg[:, 0, 0:128], pg[:, 0, 0:128], msk_sb)
                    for idx, j in enumerate(u):
                        off, w = offs[idx]
                        first, last = (j == 0), (j == jmax - 1)
                        nc.tensor.matmul(
                            o_ps[:, off:512],
                            v_sb[:, j, 128 * habs:128 * habs + 128],
                            pg[:, idx, :w], start=first, stop=last)

                # staggered two-head pipeline: each S-group waits on an exp
                # issued a full unit earlier, with weave points in between
                s0g, offs0 = emit_s(units[0], 0)
                for n_u, u in enumerate(units):
                    s1g, offs1 = emit_s(u, 1)
                    consume(u, 0, s0g, offs0, o0, h0)
                    ui += 1
                    pop_weave()
                    if n_u + 1 < len(units):
                        s0g, offs0 = emit_s(units[n_u + 1], 0)
                    consume(u, 1, s1g, offs1, o1, h1)
                    pop_weave()

                cs = slice(c * 512, (c + 1) * 512)
                for hh, o_ps in ((0, o0), (1, o1)):
                    rr = slice(hh * 64, hh * 64 + 64)
                    rcp = div_pool.tile([64, 512], F32, tag="rcp")
                    nc.vector.reciprocal(rcp, o_ps[64:128, :])
                    nc.vector.tensor_mul(ot_sb[pr][rr, cs], o_ps[0:64, :], rcp)
                if pr == NJT - 1:
                    # this sc's column of the output projection is now final
                    weave.extend(
                        (lambda mt=mt, sc=c: wo_unit(mt, sc))
                        for mt in range(D // 128)
                    )
            # drain any unwoven closures
            while wi < len(weave):
                weave[wi]()
                wi += 1

        # ---- prologue: V units and pair-0 projections interleaved ----
        proj0 = make_pair_proj_closures(0)
        pi = 0
        for sc in range(NSC):
            for st4 in range(4):
                v_unit(sc, st4)
                while pi * 16 < (sc * 4 + st4 + 1) * len(proj0):
                    proj0[pi]()
                    pi += 1
        while pi < len(proj0):
            proj0[pi]()
            pi += 1

        # ---- attention pairs with next pair's projections woven in ----
        for pr in range(NJT):
            weave = make_pair_proj_closures(pr + 1) if pr + 1 < NJT else []
            attention(pr, weave)




_BUILT = {}


def _build(reps=1):
    if reps in _BUILT:
        return _BUILT[reps]
    nc = bacc.Bacc("TRN2", target_bir_lowering=False, debug=False)
    xT = nc.dram_tensor("xT", [D, S], BF, kind="ExternalInput").ap()
    wqT = nc.dram_tensor("wqT", [D, DG], BF, kind="ExternalInput").ap()
    wkT = nc.dram_tensor("wkT", [D, DG], BF, kind="ExternalInput").ap()
    wvT = nc.dram_tensor("wvT", [D, DG], BF, kind="ExternalInput").ap()
    woT = nc.dram_tensor("woT", [DG, D], BF, kind="ExternalInput").ap()
    ropeC = nc.dram_tensor("ropeC", [128, S], F32, kind="ExternalInput").ap()
    ropeS = nc.dram_tensor("ropeS", [128, S], F32, kind="ExternalInput").ap()
    rmat = nc.dram_tensor("rmat", [128, 128], BF, kind="ExternalInput").ap()
    cmask = nc.dram_tensor("cmask", [128, 128], BF, kind="ExternalInput").ap()
    out = nc.dram_tensor("out", [D, S], mybir.dt.float32, kind="ExternalOutput").ap()
    aps = (xT, wqT, wkT, wvT, woT, ropeC, ropeS, rmat, cmask, out)
    with tile.TileContext(nc) as tc:
        with ExitStack() as ctx:
            tc.ctx = ctx
            _emit(tc, aps, reps=reps)
    nc.compile()
    _BUILT[reps] = nc
    return nc


def _host_consts():
    perm64 = np.concatenate([np.arange(0, 64, 2), np.arange(1, 64, 2)])
    perm512 = np.concatenate([h * 64 + perm64 for h in range(HL)])
    invf = THETA ** (-(np.arange(32) * 2.0) / DH)
    pos = np.arange(S, dtype=np.float64)
    iofp = np.arange(128) % 32
    ang = pos[None, :] * invf[iofp][:, None]
    ropeC = np.cos(ang).astype(np.float32)
    ropeS = np.sin(ang).astype(np.float32)
    mblk = np.zeros((64, 64), np.float32)
    for i in range(32):
        mblk[i, 32 + i] = -1.0
        mblk[32 + i, i] = 1.0
    rmat = np.kron(np.eye(2, dtype=np.float32), mblk).T.astype(BF16)  # lhsT = M^T
    cmask = (np.arange(128)[None, :] >= np.arange(128)[:, None]).astype(BF16)
    return perm512, ropeC, ropeS, rmat, cmask


LAST_RESULT = None
_last_in_maps = None


def kernel(x, wq, wk, wv, wo):
    global LAST_RESULT, _last_in_maps
    x = np.asarray(x, np.float32)
    wq = np.asarray(wq, np.float32)
    wk = np.asarray(wk, np.float32)
    wv = np.asarray(wv, np.float32)
    wo = np.asarray(wo, np.float32)

    perm512, ropeC, ropeS, rmat, cmask = _host_consts()
    nc = _build()

    in_maps = []
    for core in range(8):
        b, g = core // NG, core % NG
        gsl = slice(g * DG, (g + 1) * DG)
        in_maps.append({
            "xT": np.ascontiguousarray(x[b].T).astype(BF16),
            "wqT": np.ascontiguousarray(wq[gsl][perm512].T).astype(BF16),
            "wkT": np.ascontiguousarray(wk[gsl][perm512].T).astype(BF16),
            "wvT": np.ascontiguousarray(wv[gsl].T).astype(BF16),
            "woT": np.ascontiguousarray(wo[:, gsl].T).astype(BF16),
            "ropeC": ropeC,
            "ropeS": ropeS,
            "rmat": rmat,
            "cmask": cmask,
        })

    _last_in_maps = in_maps
    # the axon NTFF profile hook is unavailable in this container; make sure
    # a stray BASS_TRACE in the environment can't route us into it
    os.environ["BASS_NEVER_TRACE"] = "1"
    res = run_bass_kernel_spmd(nc, in_maps, list(range(8)))
    LAST_RESULT = res

    out = np.empty((B, S, D), np.float32)
    for b in range(B):
        acc = res.results[2 * b]["out"].astype(np.float32) + \
            res.results[2 * b + 1]["out"].astype(np.float32)
        out[b] = acc.T
    return out



# revision 5
# speedup vs baseline: 1.0426x; 1.0426x over previous
"""Causal MHA (B=4, S=2048, D=1024, H=16, Dh=64) on 8 trn2 NeuronCores.

Sharding: core = (batch b = core//2) x (head-group g = core%2, 8 heads each).
No collectives: each core computes a partial output projection for its head
group; the host sums the two partials per batch.

On-chip layout is fully "transposed" (feature-major) so no on-chip transposes
are needed:
  - x^T [1024, 2048] is the input;  Q^T/K^T [512, 2048] come out of the
    projection with the moving operand = x^T.
  - RoPE pair-rotation is a fixed 128x128 matrix (folded per 2-head block)
    applied on the PE, plus two elementwise multiplies with cos/sin tables.
  - scores are computed directly as S^T [k, q] tiles (lhsT = K^T slice),
    softmax denominator comes for free from a ones-column appended to V.
  - attention output is O^T [d, q] (lhsT = V tile), which feeds the wo
    projection directly (lhsT = wo^T tiles).
Causality is exploited at tile granularity (only j*128 < qchunk_end k-tiles
are computed; the q-range of diagonal-band tiles is clipped; exact diagonal
128x128 blocks get a multiplicative 0/1 mask after exp).

Perf structure (vs the original version):
  - The two heads' S^T matmuls are K=64 each; they are emitted back-to-back
    into the two banks of one PSUM tile so the PE row-tiles them
    concurrently (tile_position (0,0)/(64,0) auto-derived) -> S cost ~halves.
  - One wide exp per k-tile covers both heads ([128, 2, 512]) -> half the
    ACT per-instruction overhead.
  - Softmax division uses reciprocal_approx_fast (1 pass) instead of the
    iterative-divide `reciprocal` (~8 cyc/elem), batched over both heads.
  - RoPE cos/sin tables are bf16 so 2 of the 3 DVE ops run in 2x mode.
"""
import os
from contextlib import ExitStack

import numpy as np
import ml_dtypes

import concourse.bass as bass
from concourse import bacc
import concourse.mybir as mybir
import concourse.tile as tile
from concourse.bass_utils import run_bass_kernel_spmd

BF16 = ml_dtypes.bfloat16
F32 = mybir.dt.float32
BF = mybir.dt.bfloat16

B, S, D, H, DH = 4, 2048, 1024, 16, 64
NG = 2               # head groups
HL = H // NG         # heads per core = 8
DG = HL * DH         # 512 local head dims
THETA = 10000.0
NDT = D // 128       # 8 d-tiles of x^T
NJT = DG // 128      # 4 tiles of Q^T/K^T/O^T rows
NST = S // 128       # 16 seq tiles
NSC = S // 512       # 4 seq chunks
EXPF = mybir.ActivationFunctionType.Exp
LNF = mybir.ActivationFunctionType.Ln


def _emit(tc, aps, reps=1):
    nc = tc.nc
    (xT, wqT, wkT, wvT, woT, ropeC, ropeS, rmat, cmask, out) = aps

    ctx = tc.ctx  # set by caller

    # ---------------- persistent SBUF residents ----------------
    singles = ctx.enter_context(tc.tile_pool(name="singles", bufs=1))
    wq_sb = singles.tile([128, NDT, DG], BF, tag="wq")
    wk_sb = singles.tile([128, NDT, DG], BF, tag="wk")
    wv_sb = singles.tile([128, NDT, DG], BF, tag="wv")
    wo_sb = singles.tile([128, NJT, D], BF, tag="wo")
    c_sb = singles.tile([128, S], BF, tag="ropec")
    s_sb = singles.tile([128, S], BF, tag="ropes")
    rm_sb = singles.tile([128, 128], BF, tag="rmat")
    msk_sb = singles.tile([128, 2, 128], BF, tag="cmask")
    qt_sb = [singles.tile([128, S], BF, tag=f"qt{j}", name=f"qt{j}") for j in range(NJT)]
    kt_sb = [singles.tile([128, S], BF, tag=f"kt{j}", name=f"kt{j}") for j in range(NJT)]
    ot_sb = [singles.tile([128, S], BF, tag=f"ot{j}", name=f"ot{j}") for j in range(NJT)]
    v_sb = singles.tile([128, NST, 128 * HL], BF, tag="v")

    xpool = ctx.enter_context(tc.tile_pool(name="xstream", bufs=1))
    qpre_pool = ctx.enter_context(tc.tile_pool(name="qpre", bufs=4))
    tmp_pool = ctx.enter_context(tc.tile_pool(name="ropetmp", bufs=3))
    p_pool = ctx.enter_context(tc.tile_pool(name="ptiles", bufs=3))
    div_pool = ctx.enter_context(tc.tile_pool(name="div", bufs=2))
    out_pool = ctx.enter_context(tc.tile_pool(name="outc", bufs=3))

    # loads ordered so the first compute (V units, pair-0 proj) starts early
    def load_xt(sc):
        xt = xpool.tile([128, NDT, 512], BF, tag=f"xt{sc}", name=f"xt{sc}")
        nc.sync.dma_start(
            out=xt,
            in_=xT[:, sc * 512:(sc + 1) * 512].rearrange("(t p) w -> p t w", p=128),
        )
        return xt

    xt0 = xpool.tile([128, NDT, 512], BF, tag="xt0", name="xt0")
    for dt in range(NDT):
        nc.sync.dma_start(out=xt0[:, dt, :], in_=xT[dt * 128:(dt + 1) * 128, 0:512])
        nc.sync.dma_start(
            out=wv_sb[:, dt, :], in_=wvT[dt * 128:(dt + 1) * 128, :])
        nc.sync.dma_start(
            out=wq_sb[:, dt, :], in_=wqT[dt * 128:(dt + 1) * 128, :])
    xt_tiles = [xt0]
    nc.sync.dma_start(out=rm_sb, in_=rmat[:])
    nc.sync.dma_start(out=wk_sb, in_=wkT.rearrange("(t p) j -> p t j", p=128))
    nc.sync.dma_start(out=c_sb, in_=ropeC[:])
    nc.sync.dma_start(out=s_sb, in_=ropeS[:])
    for sc in range(1, NSC):
        xt_tiles.append(load_xt(sc))
    nc.sync.dma_start(out=msk_sb, in_=cmask.rearrange("p (h m) -> p h m", h=2))
    nc.sync.dma_start(out=wo_sb, in_=woT.rearrange("(t p) m -> p t m", p=128))
    # ones half-block per head: AV matmul then yields rowsum replicated
    # on out partitions 64..127 (no partition-broadcast needed for the div)
    nc.vector.memset(
        v_sb.rearrange("p s (h c) -> p s h c", h=HL)[:, :, :, 64:128], 1.0
    )

    for _rep in range(reps):
        _phases(nc, tc, ctx, locals())


def _phases(nc, tc, ctx, env):
    (xpool, qpre_pool, tmp_pool, p_pool, div_pool, out_pool) = (
        env["xpool"], env["qpre_pool"], env["tmp_pool"], env["p_pool"],
        env["div_pool"], env["out_pool"])
    (wq_sb, wk_sb, wv_sb, wo_sb, c_sb, s_sb, rm_sb, msk_sb) = (
        env["wq_sb"], env["wk_sb"], env["wv_sb"], env["wo_sb"], env["c_sb"],
        env["s_sb"], env["rm_sb"], env["msk_sb"])
    (qt_sb, kt_sb, ot_sb, v_sb, xT, out) = (
        env["qt_sb"], env["kt_sb"], env["ot_sb"], env["v_sb"], env["xT"],
        env["out"])

    with ExitStack() as ph:
        # 8 PSUM banks exactly: pp(2x1) + sg(2x2) + o(1x2)
        psum_a = ph.enter_context(tc.tile_pool(name="psum_a", bufs=2, space="PSUM"))
        psum_s = ph.enter_context(tc.tile_pool(name="psum_s", bufs=2, space="PSUM"))
        psum_o = ph.enter_context(tc.tile_pool(name="psum_o", bufs=1, space="PSUM"))

        xt_tiles = env["xt_tiles"]

        # ---- projection unit builders (interleaved into attention) ----
        def proj_qk_mm(wsel, jt, sc):
            w_sb = wq_sb if wsel == 0 else wk_sb
            pp = psum_a.tile([128, 512], F32, tag="pp")
            for dt in range(NDT):
                nc.tensor.matmul(
                    pp, w_sb[:, dt, jt * 128:(jt + 1) * 128],
                    xt_tiles[sc][:, dt, :],
                    start=(dt == 0), stop=(dt == NDT - 1),
                )
            qpre = qpre_pool.tile([128, 512], BF, tag="qpre")
            nc.any.tensor_copy(qpre, pp)
            return qpre

        def proj_qk_rope(qpre, wsel, pr, sc):
            dst = qt_sb if wsel == 0 else kt_sb
            rq = psum_a.tile([128, 512], F32, tag="pp")
            nc.tensor.matmul(rq, rm_sb, qpre, start=True, stop=True)
            t1 = tmp_pool.tile([128, 512], BF, tag="t1")
            t2 = tmp_pool.tile([128, 512], BF, tag="t2")
            cs = slice(sc * 512, (sc + 1) * 512)
            nc.vector.tensor_mul(t1, qpre, c_sb[:, cs])
            nc.vector.tensor_mul(t2, rq, s_sb[:, cs])
            nc.vector.tensor_add(dst[pr][:, cs], t1, t2)

        def make_pair_proj_closures(pr):
            clos = []
            for sc in range(NSC):
                for wsel in (0, 1):
                    def mk(wsel=wsel, sc=sc):
                        st = {}
                        def a():
                            st["qpre"] = proj_qk_mm(wsel, pr, sc)
                        def b():
                            proj_qk_rope(st["qpre"], wsel, pr, sc)
                        return a, b
                    a, b = mk()
                    clos.append(a)
                    clos.append(b)
            return clos

        def v_unit(sc, st4):
            st = sc * 4 + st4
            vp = psum_a.tile([128, 512], F32, tag="pp")
            for dt in range(NDT):
                nc.tensor.matmul(
                    vp, xt_tiles[sc][:, dt, st4 * 128:(st4 + 1) * 128],
                    wv_sb[:, dt, :],
                    start=(dt == 0), stop=(dt == NDT - 1),
                )
            nc.any.tensor_copy(
                v_sb[:, st, :].rearrange("p (h c) -> p h c", h=HL)[:, :, 0:64],
                vp.rearrange("p (h c) -> p h c", h=HL),
            )

        def wo_unit(mt, sc):
            wp = psum_a.tile([128, 512], F32, tag="pp", name="wp")
            for jt in range(NJT):
                nc.tensor.matmul(
                    wp, wo_sb[:, jt, mt * 128:(mt + 1) * 128],
                    ot_sb[jt][:, sc * 512:(sc + 1) * 512],
                    start=(jt == 0), stop=(jt == NJT - 1),
                )
            ob = out_pool.tile([128, 512], F32, tag="ob", name="ob")
            nc.any.tensor_copy(ob, wp)
            nc.sync.dma_start(
                out=out[mt * 128:(mt + 1) * 128, sc * 512:(sc + 1) * 512],
                in_=ob,
            )

        # ---- attention for one head pair, with proj closures woven in ----
        def attention(pr, weave):
            wi = 0          # closures popped
            ui = 0          # attention units emitted
            n_units = sum(4 * c + 4 for c in range(NSC))

            def pop_weave():
                nonlocal wi
                want = (ui * len(weave)) // max(1, n_units)
                while wi < min(want, len(weave)):
                    weave[wi]()
                    wi += 1

            h0, h1 = 2 * pr, 2 * pr + 1
            for c in range(NSC):
                jmax = 4 * c + 4
                o_ps = psum_o.tile([128, 2, 512], F32, tag="o")

                def emit_s2(j, c=c, pr=pr):
                    # both heads' K=64 S-matmuls back-to-back: the PE
                    # row-tiles them concurrently (positions (0,0)/(64,0))
                    sg = psum_s.tile([128, 2, 512], F32, tag="sg")
                    off = max(0, j * 128 - c * 512)
                    w = 512 - off
                    qs = slice(c * 512 + off, (c + 1) * 512)
                    ks = slice(j * 128, (j + 1) * 128)
                    nc.tensor.matmul(
                        sg[:, 0, :w], kt_sb[pr][0:64, ks],
                        qt_sb[pr][0:64, qs], start=True, stop=True)
                    nc.tensor.matmul(
                        sg[:, 1, :w], kt_sb[pr][64:128, ks],
                        qt_sb[pr][64:128, qs], start=True, stop=True)
                    return sg, off, w

                def consume_exp(j, sg, off, w, c=c):
                    # one wide exp covers both heads' banks
                    pg = p_pool.tile([128, 2, 512], BF, tag="pg")
                    nc.scalar.activation(
                        pg[:, :, :w], sg[:, :, :w], EXPF, scale=0.125)
                    if j * 128 >= c * 512:
                        nc.vector.tensor_mul(
                            pg[:, :, 0:128], pg[:, :, 0:128], msk_sb)
                    return pg

                def consume_av(j, pg, off, w, o_ps=o_ps, jmax=jmax):
                    first, last = (j == 0), (j == jmax - 1)
                    for hh in (0, 1):
                        nc.tensor.matmul(
                            o_ps[:, hh, off:512],
                            v_sb[:, j, 128 * (2 * pr + hh):128 * (2 * pr + hh) + 128],
                            pg[:, hh, :w], start=first, stop=last)

                sgd = emit_s2(0)
                for j in range(jmax):
                    cur_sg, cur_off, cur_w = sgd
                    pg = consume_exp(j, cur_sg, cur_off, cur_w)
                    if j + 1 < jmax:
                        sgd = emit_s2(j + 1)
                    ui += 1
                    pop_weave()
                    consume_av(j, pg, cur_off, cur_w)
                    pop_weave()

                cs = slice(c * 512, (c + 1) * 512)
                # evacuate o in ONE op so the banks free fast (next chunk's
                # first AV is behind this in the strict-FIFO PE queue);
                # softmax 1/d as exp(-ln(d)) on ACT (same table set as the
                # attention exps) -- DVE `reciprocal` is ~8 cyc/elem.
                oc = div_pool.tile([128, 2, 512], BF, tag="oc")
                nc.vector.tensor_copy(oc, o_ps)
                lg = div_pool.tile([64, 2, 512], F32, tag="lg")
                nc.scalar.activation(lg, oc[64:128, :, :], LNF)
                rcp = div_pool.tile([64, 2, 512], BF, tag="rcp")
                nc.scalar.activation(rcp, lg, EXPF, scale=-1.0)
                nc.vector.tensor_mul(
                    ot_sb[pr][0:64, cs], oc[0:64, 0, :], rcp[:, 0, :])
                nc.vector.tensor_mul(
                    ot_sb[pr][64:128, cs], oc[0:64, 1, :], rcp[:, 1, :])
                if pr == NJT - 1:
                    # this sc's column of the output projection is now final
                    weave.extend(
                        (lambda mt=mt, sc=c: wo_unit(mt, sc))
                        for mt in range(D // 128)
                    )
            # drain any unwoven closures
            while wi < len(weave):
                weave[wi]()
                wi += 1

        # ---- prologue: V units and pair-0 projections interleaved ----
        proj0 = make_pair_proj_closures(0)
        pi = 0
        for sc in range(NSC):
            for st4 in range(4):
                v_unit(sc, st4)
                while pi * 16 < (sc * 4 + st4 + 1) * len(proj0):
                    proj0[pi]()
                    pi += 1
        while pi < len(proj0):
            proj0[pi]()
            pi += 1

        # ---- attention pairs with next pair's projections woven in ----
        for pr in range(NJT):
            weave = make_pair_proj_closures(pr + 1) if pr + 1 < NJT else []
            attention(pr, weave)




_BUILT = {}


def _build(reps=1):
    if reps in _BUILT:
        return _BUILT[reps]
    nc = bacc.Bacc("TRN2", target_bir_lowering=False, debug=False)
    xT = nc.dram_tensor("xT", [D, S], BF, kind="ExternalInput").ap()
    wqT = nc.dram_tensor("wqT", [D, DG], BF, kind="ExternalInput").ap()
    wkT = nc.dram_tensor("wkT", [D, DG], BF, kind="ExternalInput").ap()
    wvT = nc.dram_tensor("wvT", [D, DG], BF, kind="ExternalInput").ap()
    woT = nc.dram_tensor("woT", [DG, D], BF, kind="ExternalInput").ap()
    ropeC = nc.dram_tensor("ropeC", [128, S], BF, kind="ExternalInput").ap()
    ropeS = nc.dram_tensor("ropeS", [128, S], BF, kind="ExternalInput").ap()
    rmat = nc.dram_tensor("rmat", [128, 128], BF, kind="ExternalInput").ap()
    cmask = nc.dram_tensor("cmask", [128, 256], BF, kind="ExternalInput").ap()
    out = nc.dram_tensor("out", [D, S], mybir.dt.float32, kind="ExternalOutput").ap()
    aps = (xT, wqT, wkT, wvT, woT, ropeC, ropeS, rmat, cmask, out)
    with tile.TileContext(nc) as tc:
        with ExitStack() as ctx:
            tc.ctx = ctx
            _emit(tc, aps, reps=reps)
    nc.compile()
    _BUILT[reps] = nc
    return nc


def _host_consts():
    perm64 = np.concatenate([np.arange(0, 64, 2), np.arange(1, 64, 2)])
    perm512 = np.concatenate([h * 64 + perm64 for h in range(HL)])
    invf = THETA ** (-(np.arange(32) * 2.0) / DH)
    pos = np.arange(S, dtype=np.float64)
    iofp = np.arange(128) % 32
    ang = pos[None, :] * invf[iofp][:, None]
    ropeC = np.cos(ang).astype(BF16)
    ropeS = np.sin(ang).astype(BF16)
    mblk = np.zeros((64, 64), np.float32)
    for i in range(32):
        mblk[i, 32 + i] = -1.0
        mblk[32 + i, i] = 1.0
    rmat = np.kron(np.eye(2, dtype=np.float32), mblk).T.astype(BF16)  # lhsT = M^T
    cm1 = (np.arange(128)[None, :] >= np.arange(128)[:, None]).astype(BF16)
    cmask = np.concatenate([cm1, cm1], axis=1)
    return perm512, ropeC, ropeS, rmat, cmask


LAST_RESULT = None
_last_in_maps = None


def kernel(x, wq, wk, wv, wo):
    global LAST_RESULT, _last_in_maps
    x = np.asarray(x, np.float32)
    wq = np.asarray(wq, np.float32)
    wk = np.asarray(wk, np.float32)
    wv = np.asarray(wv, np.float32)
    wo = np.asarray(wo, np.float32)

    perm512, ropeC, ropeS, rmat, cmask = _host_consts()
    nc = _build()

    in_maps = []
    for core in range(8):
        b, g = core // NG, core % NG
        gsl = slice(g * DG, (g + 1) * DG)
        in_maps.append({
            "xT": np.ascontiguousarray(x[b].T).astype(BF16),
            "wqT": np.ascontiguousarray(wq[gsl][perm512].T).astype(BF16),
            "wkT": np.ascontiguousarray(wk[gsl][perm512].T).astype(BF16),
            "wvT": np.ascontiguousarray(wv[gsl].T).astype(BF16),
            "woT": np.ascontiguousarray(wo[:, gsl].T).astype(BF16),
            "ropeC": ropeC,
            "ropeS": ropeS,
            "rmat": rmat,
            "cmask": cmask,
        })

    _last_in_maps = in_maps
    # the axon NTFF profile hook is unavailable in this container; make sure
    # a stray BASS_TRACE in the environment can't route us into it
    os.environ["BASS_NEVER_TRACE"] = "1"
    res = run_bass_kernel_spmd(nc, in_maps, list(range(8)))
    LAST_RESULT = res

    out = np.empty((B, S, D), np.float32)
    for b in range(B):
        acc = res.results[2 * b]["out"].astype(np.float32) + \
            res.results[2 * b + 1]["out"].astype(np.float32)
        out[b] = acc.T
    return out


# revision 7
# speedup vs baseline: 1.2060x; 1.1567x over previous
"""Causal MHA (B=4, S=2048, D=1024, H=16, Dh=64) on 8 trn2 NeuronCores.

Sharding: core = (batch b = core//2) x (head-group g = core%2, 8 heads each).
No collectives: each core computes a partial output projection for its head
group; the host sums the two partials per batch.

On-chip layout is fully "transposed" (feature-major) so no on-chip transposes
are needed:
  - x^T [1024, 2048] is the input;  Q^T/K^T [512, 2048] come out of the
    projection with the moving operand = x^T.
  - RoPE pair-rotation is a fixed 128x128 matrix (folded per 2-head block)
    applied on the PE, plus two elementwise multiplies with cos/sin tables.
  - scores are computed directly as S^T [k, q] tiles (lhsT = K^T slice),
    softmax denominator comes for free from a ones-column appended to V.
  - attention output is O^T [d, q] (lhsT = V tile), which feeds the wo
    projection directly (lhsT = wo^T tiles).
Causality is exploited at tile granularity (only j*128 < qchunk_end k-tiles
are computed; the q-range of diagonal-band tiles is clipped; exact diagonal
128x128 blocks get a multiplicative 0/1 mask after exp).

Perf structure (vs the original version):
  - The two heads' S^T matmuls are K=64 each; they are emitted back-to-back
    into the two banks of one PSUM tile so the PE row-tiles them
    concurrently (tile_position (0,0)/(64,0) auto-derived) -> S cost ~halves.
  - One wide exp per k-tile covers both heads ([128, 2, 512]) -> half the
    ACT per-instruction overhead.
  - Softmax division uses reciprocal_approx_fast (1 pass) instead of the
    iterative-divide `reciprocal` (~8 cyc/elem), batched over both heads.
  - RoPE cos/sin tables are bf16 so 2 of the 3 DVE ops run in 2x mode.
"""
import os
from contextlib import ExitStack

import numpy as np
import ml_dtypes

import concourse.bass as bass
from concourse import bacc
import concourse.mybir as mybir
import concourse.tile as tile
from concourse.bass_utils import run_bass_kernel_spmd

BF16 = ml_dtypes.bfloat16
F32 = mybir.dt.float32
BF = mybir.dt.bfloat16

B, S, D, H, DH = 4, 2048, 1024, 16, 64
NG = 2               # head groups
HL = H // NG         # heads per core = 8
DG = HL * DH         # 512 local head dims
THETA = 10000.0
NDT = D // 128       # 8 d-tiles of x^T
NJT = DG // 128      # 4 tiles of Q^T/K^T/O^T rows
NST = S // 128       # 16 seq tiles
NSC = S // 512       # 4 seq chunks
EXPF = mybir.ActivationFunctionType.Exp
LNF = mybir.ActivationFunctionType.Ln


def _emit(tc, aps, reps=1):
    nc = tc.nc
    (xT, wqT, wkT, wvT, woT, ropeC, ropeS, rmat, cmask, out) = aps

    ctx = tc.ctx  # set by caller

    # ---------------- persistent SBUF residents ----------------
    singles = ctx.enter_context(tc.tile_pool(name="singles", bufs=1))
    wq_sb = singles.tile([128, NDT, DG], BF, tag="wq")
    wk_sb = singles.tile([128, NDT, DG], BF, tag="wk")
    wv_sb = singles.tile([128, NDT, DG], BF, tag="wv")
    wo_sb = singles.tile([128, NJT, D], BF, tag="wo")
    c_sb = singles.tile([128, S], BF, tag="ropec")
    s_sb = singles.tile([128, S], BF, tag="ropes")
    rm_sb = singles.tile([128, 128], BF, tag="rmat")
    msk_sb = singles.tile([128, 2, 128], BF, tag="cmask")
    qt_sb = [singles.tile([128, S], BF, tag=f"qt{j}", name=f"qt{j}") for j in range(NJT)]
    kt_sb = [singles.tile([128, S], BF, tag=f"kt{j}", name=f"kt{j}") for j in range(NJT)]
    ot_sb = [singles.tile([128, S], BF, tag=f"ot{j}", name=f"ot{j}") for j in range(NJT)]
    v_sb = singles.tile([128, NST, 128 * HL], BF, tag="v")

    xpool = ctx.enter_context(tc.tile_pool(name="xstream", bufs=1))
    qpre_pool = ctx.enter_context(tc.tile_pool(name="qpre", bufs=4))
    tmp_pool = ctx.enter_context(tc.tile_pool(name="ropetmp", bufs=3))
    p_pool = ctx.enter_context(tc.tile_pool(name="ptiles", bufs=3))
    div_pool = ctx.enter_context(tc.tile_pool(name="div", bufs=2))
    out_pool = ctx.enter_context(tc.tile_pool(name="outc", bufs=3))

    # loads ordered so the first compute (V units, pair-0 proj) starts early
    def load_xt(sc):
        xt = xpool.tile([128, NDT, 512], BF, tag=f"xt{sc}", name=f"xt{sc}")
        nc.sync.dma_start(
            out=xt,
            in_=xT[:, sc * 512:(sc + 1) * 512].rearrange("(t p) w -> p t w", p=128),
        )
        return xt

    xt0 = xpool.tile([128, NDT, 512], BF, tag="xt0", name="xt0")
    for dt in range(NDT):
        nc.sync.dma_start(out=xt0[:, dt, :], in_=xT[dt * 128:(dt + 1) * 128, 0:512])
        nc.sync.dma_start(
            out=wv_sb[:, dt, :], in_=wvT[dt * 128:(dt + 1) * 128, :])
        nc.sync.dma_start(
            out=wq_sb[:, dt, :], in_=wqT[dt * 128:(dt + 1) * 128, :])
    xt_tiles = [xt0]
    nc.sync.dma_start(out=rm_sb, in_=rmat[:])
    nc.sync.dma_start(out=wk_sb, in_=wkT.rearrange("(t p) j -> p t j", p=128))
    nc.sync.dma_start(out=c_sb, in_=ropeC[:])
    nc.sync.dma_start(out=s_sb, in_=ropeS[:])
    for sc in range(1, NSC):
        xt_tiles.append(load_xt(sc))
    nc.sync.dma_start(out=msk_sb, in_=cmask.rearrange("p (h m) -> p h m", h=2))
    nc.sync.dma_start(out=wo_sb, in_=woT.rearrange("(t p) m -> p t m", p=128))
    # ones half-block per head: AV matmul then yields rowsum replicated
    # on out partitions 64..127 (no partition-broadcast needed for the div)
    nc.vector.memset(
        v_sb.rearrange("p s (h c) -> p s h c", h=HL)[:, :, :, 64:128], 1.0
    )

    for _rep in range(reps):
        _phases(nc, tc, ctx, locals())


def _phases(nc, tc, ctx, env):
    (xpool, qpre_pool, tmp_pool, p_pool, div_pool, out_pool) = (
        env["xpool"], env["qpre_pool"], env["tmp_pool"], env["p_pool"],
        env["div_pool"], env["out_pool"])
    (wq_sb, wk_sb, wv_sb, wo_sb, c_sb, s_sb, rm_sb, msk_sb) = (
        env["wq_sb"], env["wk_sb"], env["wv_sb"], env["wo_sb"], env["c_sb"],
        env["s_sb"], env["rm_sb"], env["msk_sb"])
    (qt_sb, kt_sb, ot_sb, v_sb, xT, out) = (
        env["qt_sb"], env["kt_sb"], env["ot_sb"], env["v_sb"], env["xT"],
        env["out"])

    with ExitStack() as ph:
        # 8 PSUM banks exactly: pp(2x1) + sg(2x2) + o(1x2)
        psum_a = ph.enter_context(tc.tile_pool(name="psum_a", bufs=2, space="PSUM"))
        psum_s = ph.enter_context(tc.tile_pool(name="psum_s", bufs=2, space="PSUM"))
        psum_o = ph.enter_context(tc.tile_pool(name="psum_o", bufs=1, space="PSUM"))

        xt_tiles = env["xt_tiles"]

        # ---- projection unit builders (interleaved into attention) ----
        def proj_qk_mm(wsel, jt, sc):
            w_sb = wq_sb if wsel == 0 else wk_sb
            pp = psum_a.tile([128, 512], F32, tag="pp")
            for dt in range(NDT):
                nc.tensor.matmul(
                    pp, w_sb[:, dt, jt * 128:(jt + 1) * 128],
                    xt_tiles[sc][:, dt, :],
                    start=(dt == 0), stop=(dt == NDT - 1),
                )
            qpre = qpre_pool.tile([128, 512], BF, tag="qpre")
            nc.any.tensor_copy(qpre, pp)
            return qpre

        def proj_qk_rope(qpre, wsel, pr, sc):
            dst = qt_sb if wsel == 0 else kt_sb
            rq = psum_a.tile([128, 512], F32, tag="pp")
            nc.tensor.matmul(rq, rm_sb, qpre, start=True, stop=True)
            t1 = tmp_pool.tile([128, 512], BF, tag="t1")
            t2 = tmp_pool.tile([128, 512], BF, tag="t2")
            cs = slice(sc * 512, (sc + 1) * 512)
            nc.vector.tensor_mul(t1, qpre, c_sb[:, cs])
            nc.vector.tensor_mul(t2, rq, s_sb[:, cs])
            nc.vector.tensor_add(dst[pr][:, cs], t1, t2)

        def make_pair_proj_closures(pr):
            clos = []
            for sc in range(NSC):
                for wsel in (0, 1):
                    def mk(wsel=wsel, sc=sc):
                        st = {}
                        def a():
                            st["qpre"] = proj_qk_mm(wsel, pr, sc)
                        def b():
                            proj_qk_rope(st["qpre"], wsel, pr, sc)
                        return a, b
                    a, b = mk()
                    clos.append(a)
                    clos.append(b)
            return clos

        def v_unit(sc, st4):
            st = sc * 4 + st4
            vp = psum_a.tile([128, 512], F32, tag="pp")
            for dt in range(NDT):
                nc.tensor.matmul(
                    vp, xt_tiles[sc][:, dt, st4 * 128:(st4 + 1) * 128],
                    wv_sb[:, dt, :],
                    start=(dt == 0), stop=(dt == NDT - 1),
                )
            nc.any.tensor_copy(
                v_sb[:, st, :].rearrange("p (h c) -> p h c", h=HL)[:, :, 0:64],
                vp.rearrange("p (h c) -> p h c", h=HL),
            )

        def wo_unit(mt, sc):
            wp = psum_a.tile([128, 512], F32, tag="pp", name="wp")
            for jt in range(NJT):
                nc.tensor.matmul(
                    wp, wo_sb[:, jt, mt * 128:(mt + 1) * 128],
                    ot_sb[jt][:, sc * 512:(sc + 1) * 512],
                    start=(jt == 0), stop=(jt == NJT - 1),
                )
            ob = out_pool.tile([128, 512], F32, tag="ob", name="ob")
            nc.any.tensor_copy(ob, wp)
            nc.sync.dma_start(
                out=out[mt * 128:(mt + 1) * 128, sc * 512:(sc + 1) * 512],
                in_=ob,
            )

        # ---- attention for one head pair, with proj closures woven in ----
        def attention(pr, weave):
            wi = 0          # closures popped
            ui = 0          # attention units emitted
            n_units = sum(4 * c + 4 for c in range(NSC))

            def pop_weave():
                nonlocal wi
                want = (ui * len(weave)) // max(1, n_units)
                while wi < min(want, len(weave)):
                    weave[wi]()
                    wi += 1

            h0, h1 = 2 * pr, 2 * pr + 1
            for c in range(NSC):
                jmax = 4 * c + 4
                o_ps = psum_o.tile([128, 2, 512], F32, tag="o")

                def emit_s2(j, c=c, pr=pr):
                    # both heads' K=64 S-matmuls back-to-back: the PE
                    # row-tiles them concurrently (positions (0,0)/(64,0))
                    sg = psum_s.tile([128, 2, 512], F32, tag="sg")
                    off = max(0, j * 128 - c * 512)
                    w = 512 - off
                    qs = slice(c * 512 + off, (c + 1) * 512)
                    ks = slice(j * 128, (j + 1) * 128)
                    nc.tensor.matmul(
                        sg[:, 0, :w], kt_sb[pr][0:64, ks],
                        qt_sb[pr][0:64, qs], start=True, stop=True)
                    nc.tensor.matmul(
                        sg[:, 1, :w], kt_sb[pr][64:128, ks],
                        qt_sb[pr][64:128, qs], start=True, stop=True)
                    return sg, off, w

                def consume_exp(j, sg, off, w, c=c):
                    # one wide exp covers both heads' banks
                    pg = p_pool.tile([128, 2, 512], BF, tag="pg")
                    nc.scalar.activation(
                        pg[:, :, :w], sg[:, :, :w], EXPF, scale=0.125)
                    if j * 128 >= c * 512:
                        nc.vector.tensor_mul(
                            pg[:, :, 0:128], pg[:, :, 0:128], msk_sb)
                    return pg

                def consume_av(j, pg, off, w, o_ps=o_ps, jmax=jmax):
                    first, last = (j == 0), (j == jmax - 1)
                    for hh in (0, 1):
                        nc.tensor.matmul(
                            o_ps[:, hh, off:512],
                            v_sb[:, j, 128 * (2 * pr + hh):128 * (2 * pr + hh) + 128],
                            pg[:, hh, :w], start=first, stop=last)

                sgd = emit_s2(0)
                for j in range(jmax):
                    cur_sg, cur_off, cur_w = sgd
                    pg = consume_exp(j, cur_sg, cur_off, cur_w)
                    if j + 1 < jmax:
                        sgd = emit_s2(j + 1)
                    ui += 1
                    pop_weave()
                    consume_av(j, pg, cur_off, cur_w)
                    pop_weave()

                cs = slice(c * 512, (c + 1) * 512)
                # evacuate o in ONE op so the banks free fast (next chunk's
                # first AV is behind this in the strict-FIFO PE queue);
                # softmax 1/d as exp(-ln(d)) on ACT (same table set as the
                # attention exps) -- DVE `reciprocal` is ~8 cyc/elem.
                oc = div_pool.tile([128, 2, 512], BF, tag="oc")
                nc.vector.tensor_copy(oc, o_ps)
                lg = div_pool.tile([64, 2, 512], F32, tag="lg")
                nc.scalar.activation(lg, oc[64:128, :, :], LNF)
                rcp = div_pool.tile([64, 2, 512], BF, tag="rcp")
                nc.scalar.activation(rcp, lg, EXPF, scale=-1.0)
                nc.vector.tensor_mul(
                    ot_sb[pr][0:64, cs], oc[0:64, 0, :], rcp[:, 0, :])
                nc.vector.tensor_mul(
                    ot_sb[pr][64:128, cs], oc[0:64, 1, :], rcp[:, 1, :])
                if pr == NJT - 1:
                    # this sc's column of the output projection is now final
                    weave.extend(
                        (lambda mt=mt, sc=c: wo_unit(mt, sc))
                        for mt in range(D // 128)
                    )
            # drain any unwoven closures
            while wi < len(weave):
                weave[wi]()
                wi += 1

        # ---- prologue: V units and pair-0 projections interleaved ----
        proj0 = make_pair_proj_closures(0)
        pi = 0
        for sc in range(NSC):
            for st4 in range(4):
                v_unit(sc, st4)
                while pi * 16 < (sc * 4 + st4 + 1) * len(proj0):
                    proj0[pi]()
                    pi += 1
        while pi < len(proj0):
            proj0[pi]()
            pi += 1

        # ---- attention pairs with next pair's projections woven in ----
        for pr in range(NJT):
            weave = make_pair_proj_closures(pr + 1) if pr + 1 < NJT else []
            attention(pr, weave)




_BUILT = {}


def _steer_act_tables():
    """Make the act-table pass map `exp` to the set that also holds `ln`
    (`natural_log_exp_and_others`), so the per-chunk softmax `ln` doesn't
    ping-pong table loads (~2.7us each) against the attention `exp`s.
    Set names/order stay canonical; only the exp membership is narrowed,
    which is semantically valid (exp really is in the natural_log set)."""
    import concourse.bacc as _bacc_mod

    orig = _bacc_mod.get_activation_tables

    def patched(arch):
        tabs = orig(arch)
        E = mybir.ActivationFunctionType.Exp
        if any("natural_log" in n and E in f for n, f in tabs.items()):
            tabs = {
                n: (f if "natural_log" in n else (f - {E}))
                for n, f in tabs.items()
            }
        return tabs

    _bacc_mod.get_activation_tables = patched
    return lambda: setattr(_bacc_mod, "get_activation_tables", orig)


def _build(reps=1):
    if reps in _BUILT:
        return _BUILT[reps]
    nc = bacc.Bacc("TRN2", target_bir_lowering=False, debug=False)
    xT = nc.dram_tensor("xT", [D, S], BF, kind="ExternalInput").ap()
    wqT = nc.dram_tensor("wqT", [D, DG], BF, kind="ExternalInput").ap()
    wkT = nc.dram_tensor("wkT", [D, DG], BF, kind="ExternalInput").ap()
    wvT = nc.dram_tensor("wvT", [D, DG], BF, kind="ExternalInput").ap()
    woT = nc.dram_tensor("woT", [DG, D], BF, kind="ExternalInput").ap()
    ropeC = nc.dram_tensor("ropeC", [128, S], BF, kind="ExternalInput").ap()
    ropeS = nc.dram_tensor("ropeS", [128, S], BF, kind="ExternalInput").ap()
    rmat = nc.dram_tensor("rmat", [128, 128], BF, kind="ExternalInput").ap()
    cmask = nc.dram_tensor("cmask", [128, 256], BF, kind="ExternalInput").ap()
    out = nc.dram_tensor("out", [D, S], mybir.dt.float32, kind="ExternalOutput").ap()
    aps = (xT, wqT, wkT, wvT, woT, ropeC, ropeS, rmat, cmask, out)
    restore = _steer_act_tables()
    try:
        with tile.TileContext(nc) as tc:
            with ExitStack() as ctx:
                tc.ctx = ctx
                _emit(tc, aps, reps=reps)
        nc.compile()
    finally:
        restore()
    _BUILT[reps] = nc
    return nc


def _host_consts():
    perm64 = np.concatenate([np.arange(0, 64, 2), np.arange(1, 64, 2)])
    perm512 = np.concatenate([h * 64 + perm64 for h in range(HL)])
    invf = THETA ** (-(np.arange(32) * 2.0) / DH)
    pos = np.arange(S, dtype=np.float64)
    iofp = np.arange(128) % 32
    ang = pos[None, :] * invf[iofp][:, None]
    ropeC = np.cos(ang).astype(BF16)
    ropeS = np.sin(ang).astype(BF16)
    mblk = np.zeros((64, 64), np.float32)
    for i in range(32):
        mblk[i, 32 + i] = -1.0
        mblk[32 + i, i] = 1.0
    rmat = np.kron(np.eye(2, dtype=np.float32), mblk).T.astype(BF16)  # lhsT = M^T
    cm1 = (np.arange(128)[None, :] >= np.arange(128)[:, None]).astype(BF16)
    cmask = np.concatenate([cm1, cm1], axis=1)
    return perm512, ropeC, ropeS, rmat, cmask


LAST_RESULT = None
_last_in_maps = None


def kernel(x, wq, wk, wv, wo):
    global LAST_RESULT, _last_in_maps
    x = np.asarray(x, np.float32)
    wq = np.asarray(wq, np.float32)
    wk = np.asarray(wk, np.float32)
    wv = np.asarray(wv, np.float32)
    wo = np.asarray(wo, np.float32)

    perm512, ropeC, ropeS, rmat, cmask = _host_consts()
    nc = _build()

    in_maps = []
    for core in range(8):
        b, g = core // NG, core % NG
        gsl = slice(g * DG, (g + 1) * DG)
        in_maps.append({
            "xT": np.ascontiguousarray(x[b].T).astype(BF16),
            "wqT": np.ascontiguousarray(wq[gsl][perm512].T).astype(BF16),
            "wkT": np.ascontiguousarray(wk[gsl][perm512].T).astype(BF16),
            "wvT": np.ascontiguousarray(wv[gsl].T).astype(BF16),
            "woT": np.ascontiguousarray(wo[:, gsl].T).astype(BF16),
            "ropeC": ropeC,
            "ropeS": ropeS,
            "rmat": rmat,
            "cmask": cmask,
        })

    _last_in_maps = in_maps
    # the axon NTFF profile hook is unavailable in this container; make sure
    # a stray BASS_TRACE in the environment can't route us into it
    os.environ["BASS_NEVER_TRACE"] = "1"
    res = run_bass_kernel_spmd(nc, in_maps, list(range(8)))
    LAST_RESULT = res

    out = np.empty((B, S, D), np.float32)
    for b in range(B):
        acc = res.results[2 * b]["out"].astype(np.float32) + \
            res.results[2 * b + 1]["out"].astype(np.float32)
        out[b] = acc.T
    return out
